# revision 38
# baseline (speedup 1.0000x reference)
"""Trainium2 Bass kernel for nn_LogicLayer (differentiable logic-gate layer).

Reference computation:
    a = x[:, idx_a]; b = x[:, idx_b]                  # [B, OUT] gathers
    w = softmax(weights, -1)                          # [OUT, 16]
    out = sum_k w[:, k] * gate_k(a, b)

Every gate value is of the form c0 + c1*a + c2*b + c3*a*b, so
    out[i, j] = W0[j] + W1[j]*a + W2[j]*b + W3[j]*a*b
with W = softmax(weights) @ C, C the [16, 4] gate-coefficient table.

Sharding: out_dim-parallel across 8 cores (1024 neurons each, full 2048
batch), x feature-major (transposed) in bf16 and replicated. This keeps
the SWDGE gather descriptor count per core at 2048 (the Q7 descriptor
loop costs ~8.5 ns/index, so batch-parallel's 16K indices/core was the
bottleneck) while HBM traffic stays 8 MiB gathers + 8 MiB stores/core.

Device pipeline per core:
  - W = softmax(weights) @ C on ACT/DVE (overlaps the first gathers)
  - 4 SWDGE dma_gathers, each pulling 256 neurons' a- and b-columns
    (512 indices x 4 KiB bf16 rows, neurons land on partitions)
  - per 128-neuron slot: u = W3*a + W2 (ACT), v = W1*a + W0 (DVE),
    t = u*b (DVE), pre = t + v (DVE), all bf16
  - PE transposes pre back to batch-major (identity matmuls, bf16 PSUM),
    ACT/DVE copy-cast to f32, HWDGE stores [128, 512] blocks.
"""

import numpy as np

# ---------------------------------------------------------------- constants
B_TOT, IN_DIM, OUT_DIM = 2048, 8192, 8192
NCORES = 8
OC = OUT_DIM // NCORES       # 1024 neurons per core
NG = 4                       # gather groups per core (256 neurons each)
GJ = OC // NG                # 256 neurons per gather
RPT = OC // 128              # 8 (W free dim per partition)
NH = B_TOT // 128            # 16 batch blocks

# value = c0 + c1*a + c2*b + c3*ab  for each of the 16 gates
GATE_C = np.array(
    [
        # c0  c1  c2  c3
        [0, 0, 0, 0],    # 0  False
        [0, 0, 0, 1],    # 1  a AND b
        [0, 1, 0, -1],   # 2  a AND NOT b
        [0, 1, 0, 0],    # 3  a
        [0, 0, 1, -1],   # 4  NOT a AND b
        [0, 0, 1, 0],    # 5  b
        [0, 1, 1, -2],   # 6  a XOR b
        [0, 1, 1, -1],   # 7  a OR b
        [1, -1, -1, 1],  # 8  NOT (a OR b)
        [1, -1, -1, 2],  # 9  NOT (a XOR b)
        [1, 0, -1, 0],   # 10 NOT b
        [1, 0, -1, 1],   # 11 a OR NOT b
        [1, -1, 0, 0],   # 12 NOT a
        [1, -1, 0, 1],   # 13 NOT a OR b
        [1, 0, 0, -1],   # 14 NOT (a AND b)
        [1, 0, 0, 0],    # 15 True
    ],
    dtype=np.float32,
)  # [16, 4]


# ---------------------------------------------------------------- device IR
def build_nc():
    """Build the per-core Bass module (SPMD; all cores run the same IR)."""
    import sys

    if "/opt/trn_rl_repo" not in sys.path:
        sys.path.insert(0, "/opt/trn_rl_repo")

    import concourse.tile as tile
    from concourse import bacc, mybir
    from contextlib import ExitStack

    f32 = mybir.dt.float32
    bf16 = mybir.dt.bfloat16
    i16 = mybir.dt.int16
    B = B_TOT

    nc = bacc.Bacc("TRN2", target_bir_lowering=False, num_swdge_queues=2)
    xT = nc.declare_dram_parameter("xTb", [IN_DIM, B], bf16, isOutput=False)
    # one packed small-input param: cgate f32 [64] | wgt_shuf f32 [128] |
    # idxab16 i16 [128] as 64 f32 cols | identity bf16 [128] as 64 f32 cols
    # — a single DMA so the first gather's dependency chain is one transfer,
    # and no GpSimd prep work sits ahead of the gathers
    PKW = 64 + RPT * 16 + NG * 16 + 64
    pk = nc.declare_dram_parameter("pk", [128, PKW], f32, isOutput=False)
    out = nc.declare_dram_parameter("out", [B, OC], bf16, isOutput=True)

    Ident = mybir.ActivationFunctionType.Identity
    Exp = mybir.ActivationFunctionType.Exp
    MULT = mybir.AluOpType.mult
    ADD = mybir.AluOpType.add

    with tile.TileContext(nc) as tc, ExitStack() as ctx:
        # kick the Q7 ucode-library load for dma_gather as early as possible
        try:
            from concourse import library_config

            nc.gpsimd.load_library(library_config.mlp)
        except Exception:
            pass
        cpool = ctx.enter_context(tc.tile_pool(name="consts", bufs=1))
        idx_pool = ctx.enter_context(tc.tile_pool(name="idxp", bufs=1))
        wpool = ctx.enter_context(tc.tile_pool(name="wtmp", bufs=2))

        pkt = idx_pool.tile([128, PKW], f32, name="pkt")
        nc.sync.dma_start(pkt[:], pk[:])
        cgt = pkt[:, 0:64]
        wtile = pkt[:, 64:64 + RPT * 16]
        i0 = 64 + RPT * 16
        idx_sb = pkt[:, i0:i0 + NG * 16].bitcast(i16)      # [128, NG*32] i16
        identb = pkt[:, i0 + NG * 16:].bitcast(bf16)       # [128, 128] bf16

        # ---- gathers: one per 256-neuron group, a+b combined ----------
        gpool = ctx.enter_context(tc.tile_pool(name="gath", bufs=4))
        nreg = nc.gpsimd.to_reg(2 * GJ)
        gt = []
        for gk in range(NG):
            g = gpool.tile([128, 4, B], bf16, name=f"g{gk}", tag="g")
            nc.gpsimd.dma_gather(
                g[:], xT[:], idx_sb[:, gk * 32:(gk + 1) * 32], 2 * GJ, nreg, B,
                queue_num=gk % 2,
            )
            gt.append(g)

        # ---- W = softmax(weights) @ C, in (q, r) layout: j = r*128 + q ----
        wk = [cpool.tile([128, RPT], f32, name=f"wk{k}") for k in range(4)]
        wexp = wpool.tile([128, RPT * 16], f32, name="wexp")
        nc.scalar.activation(wexp[:], wtile, Exp)
        wsum = wpool.tile([128, RPT], f32, name="wsum")
        nc.vector.tensor_reduce(
            out=wsum[:],
            in_=wexp[:].rearrange("p (r k) -> p r k", k=16),
            op=ADD,
            axis=mybir.AxisListType.X,
        )
        wrcp = wpool.tile([128, RPT], f32, name="wrcp")
        nc.vector.reciprocal(wrcp[:], wsum[:])
        for k in range(4):
            wtmp = wpool.tile([128, RPT * 16], f32, name="wtmp", tag="wtmp")
            ck_bcast = (
                cgt[:, k * 16:(k + 1) * 16]
                .rearrange("p (r k) -> p r k", r=1)
                .to_broadcast([128, RPT, 16])
            )
            nc.vector.tensor_tensor(
                out=wtmp[:].rearrange("p (r k) -> p r k", k=16),
                in0=wexp[:].rearrange("p (r k) -> p r k", k=16),
                in1=ck_bcast,
                op=MULT,
            )
            wred = wpool.tile([128, RPT], f32, name="wred", tag="wred")
            nc.vector.tensor_reduce(
                out=wred[:],
                in_=wtmp[:].rearrange("p (r k) -> p r k", k=16),
                op=ADD,
                axis=mybir.AxisListType.X,
            )
            nc.vector.tensor_tensor(out=wk[k][:], in0=wred[:], in1=wrcp[:],
                                    op=MULT)

        # ---- per-slot gates + transpose-back --------------------------
        uvpool = ctx.enter_context(tc.tile_pool(name="uv", bufs=6))
        prepool = ctx.enter_context(tc.tile_pool(name="pre", bufs=10))
        psumO = ctx.enter_context(tc.tile_pool(name="psumO", bufs=4, space="PSUM"))
        ostg = ctx.enter_context(tc.tile_pool(name="ostg", bufs=4))

        pre = [None] * RPT
        for gk in range(NG):
            g = gt[gk]
            # top static priority: the scheduler's cost model underestimates
            # gather latency and would otherwise order earlier og copies
            # ahead of this compute in the engine FIFOs, stalling the tail
            with tc.high_priority():
                for cj in range(2):
                    r = gk * 2 + cj
                    u = uvpool.tile([128, B], bf16, tag="u")
                    if gk == NG - 1:
                        # last group's data arrives when ACT may still be
                        # draining copies; DVE tensor_scalar (4x mode) keeps
                        # the closing u->t->pre chain on one fast engine
                        nc.vector.tensor_scalar(
                            u[:], g[:, cj, :],
                            wk[3][:, r:r + 1], wk[2][:, r:r + 1],
                            op0=MULT, op1=ADD,
                        )
                    else:
                        nc.scalar.activation(
                            u[:], g[:, cj, :], Ident,
                            scale=wk[3][:, r:r + 1], bias=wk[2][:, r:r + 1],
                        )
                    v = uvpool.tile([128, B], bf16, tag="v")
                    nc.vector.tensor_scalar(
                        v[:], g[:, cj, :],
                        wk[1][:, r:r + 1], wk[0][:, r:r + 1],
                        op0=MULT, op1=ADD,
                    )
                    t = uvpool.tile([128, B], bf16, tag="t")
                    nc.vector.tensor_tensor(t[:], u[:], g[:, 2 + cj, :],
                                            op=MULT)
                    p = prepool.tile([128, B], bf16, tag="p")
                    nc.vector.tensor_tensor(p[:], t[:], v[:], op=ADD)
                    pre[r] = p
            if gk == 1:
                # first half (slots 0-3): 512 neurons back to batch-major;
                # two batch-blocks per PSUM bank halve the copy count, four
                # batch-blocks share one og tile and one strided DMA. Copies
                # and stores are deprioritized so the scheduler fills engine
                # gaps with them instead of delaying the gate compute.
                for hg in range(NH // 4):
                    og = ostg.tile([128, 4, 512], bf16, tag="og")
                    for i2 in range(2):
                        po = psumO.tile([128, 1024], bf16, tag="po")
                        for hh in range(2):
                            h = hg * 4 + i2 * 2 + hh
                            for s in range(4):
                                c0 = hh * 512 + s * 128
                                nc.tensor.transpose(
                                    po[:, c0:c0 + 128],
                                    pre[s][:, h * 128:(h + 1) * 128],
                                    identb,
                                )
                        dst_og = (og[:, i2 * 2:(i2 + 1) * 2, :]
                                  .rearrange("p i j -> p (i j)"))
                        with tc.high_priority(offset=-1000000):
                            if i2 == 0:
                                nc.scalar.copy(dst_og, po[:])
                            else:
                                nc.vector.tensor_copy(dst_og, po[:])
                    dst = out[hg * 512:(hg + 1) * 512, 0:512]
                    eng = nc.sync if hg % 2 == 0 else nc.scalar
                    with tc.high_priority(offset=-1000000):
                        eng.dma_start(
                            dst.rearrange("(i p) j -> p i j", p=128), og[:]
                        )
            elif gk >= 2:
                # second half in two slot-pair quarters: slots 4-5 flush
                # without waiting for the final gather's slots 6-7
                for hg in range(NH // 4):
                    og = ostg.tile([128, 4, 256], bf16, tag="ogq")
                    for i2 in range(2):
                        po = psumO.tile([128, 512], bf16, tag="poq")
                        for hh in range(2):
                            h = hg * 4 + i2 * 2 + hh
                            for s in range(2):
                                c0 = hh * 256 + s * 128
                                nc.tensor.transpose(
                                    po[:, c0:c0 + 128],
                                    pre[gk * 2 + s][:, h * 128:(h + 1) * 128],
                                    identb,
                                )
                        dst_og = (og[:, i2 * 2:(i2 + 1) * 2, :]
                                  .rearrange("p i j -> p (i j)"))
                        with tc.high_priority(offset=-1000000):
                            if i2 == 0:
                                nc.scalar.copy(dst_og, po[:])
                            else:
                                nc.vector.tensor_copy(dst_og, po[:])
                    dst = out[hg * 512:(hg + 1) * 512,
                              gk * 256:(gk + 1) * 256]
                    eng = nc.sync if hg % 2 == 0 else nc.scalar
                    with tc.high_priority(offset=-1000000):
                        eng.dma_start(
                            dst.rearrange("(i p) j -> p i j", p=128), og[:]
                        )
    nc.compile()
    return nc


# ---------------------------------------------------------------- host side
def _wrap_idx_core(ia_core, ib_core):
    """Pack one core's combined a/b indices into the wrapped int16 layout.

    Gather gk covers neurons [gk*GJ, (gk+1)*GJ) of this core's shard and
    pulls 2*GJ rows: the GJ idx_a rows then the GJ idx_b rows. Unwrapped
    position i = s*16 + p reads idx16[p % 16, gk*32 + s]; replicated over
    the 8 groups of 16 partitions.
    """
    comb = np.stack(
        [np.concatenate([ia_core[gk * GJ:(gk + 1) * GJ],
                         ib_core[gk * GJ:(gk + 1) * GJ]])
         for gk in range(NG)]
    ).astype(np.int16)  # [NG, 2*GJ]
    w = comb.reshape(NG, 32, 16).transpose(2, 0, 1).reshape(16, NG * 32)
    return np.ascontiguousarray(np.tile(w, (8, 1)))  # [128, NG*32]


def _prep_inputs(x, weights, idx_a, idx_b):
    import ml_dtypes

    x = np.asarray(x, dtype=np.float32)
    weights = np.asarray(weights, dtype=np.float32)
    idx_a = np.asarray(idx_a)
    idx_b = np.asarray(idx_b)
    xTb = np.ascontiguousarray(x.T.astype(ml_dtypes.bfloat16))  # [IN, B] bf16
    cgate = np.tile(GATE_C.T.reshape(1, 64), (128, 1))
    in_maps = []
    for c in range(NCORES):
        wc = weights[c * OC:(c + 1) * OC]              # [OC, 16]
        wgt_shuf = wc.reshape(RPT, 128, 16).transpose(1, 0, 2).reshape(128, -1)
        iw = _wrap_idx_core(idx_a[c * OC:(c + 1) * OC],
                            idx_b[c * OC:(c + 1) * OC])
        ident = np.eye(128, dtype=ml_dtypes.bfloat16).view(np.float32)
        pk = np.ascontiguousarray(
            np.concatenate(
                [cgate, wgt_shuf, iw.view(np.float32), ident], axis=1
            )
        )
        in_maps.append({"xTb": xTb, "pk": pk})
    return in_maps


_NC_CACHE = {}


def _get_nc():
    if "nc" not in _NC_CACHE:
        _NC_CACHE["nc"] = build_nc()
    return _NC_CACHE["nc"]


def kernel(x, weights, idx_a, idx_b):
    import sys

    if "/opt/trn_rl_repo" not in sys.path:
        sys.path.insert(0, "/opt/trn_rl_repo")
    from concourse.bass_utils import run_bass_kernel_spmd

    nc = _get_nc()
    in_maps = _prep_inputs(x, weights, idx_a, idx_b)
    res = run_bass_kernel_spmd(nc, in_maps, list(range(NCORES)))
    return np.concatenate(
        [r["out"].astype(np.float32) for r in res.results], axis=1
    )


if __name__ == "__main__":
    nc = build_nc()
    print("built OK")
